# revision 49
# baseline (speedup 1.0000x reference)
"""Trainium2 Bass kernel for a dense transformer encoder layer.

Model (faithful to the oracle):
  q,k,v = x@wq+bq, x@wk+bk, x@wv+bv          (12 heads, dk=64, DIM=768)
  scores = q@k^T / sqrt(768)  (note: sqrt(dim_model), not sqrt(dk))
  scores[mask==0] = 1e-11  (NOT -inf; masked keys still contribute ~1/Z)
  attn = softmax(scores); z = attn@v; o = z@wo+bo
  l1 = x + LN(o);  ffn = relu(l1@w1+b1)@w2+b2;  out = l1 + LN(ffn)

Sharding: 4096 tokens (B=2,S=2048) split 8 ways -> 512 query tokens per
core. Cores 0-3 own batch 0, cores 4-7 batch 1. K/V are computed for
the core's whole batch (redundantly within each 4-core group) so there
are NO collectives: cores run fully independently, immune to cross-core
dispatch skew.

Mask trick: the key mask is folded into K at projection time:
K_masked[:,kpos] = (K[:,kpos]+bk) * m[kpos], m in {0,1}. Masked keys
produce scores == 0 exactly and exp(0) = 1.0 == fp32(exp(1e-11)),
matching the oracle bit-for-bit in fp32. The exp scale is then a
compile-time constant, so score tiles are exp'ed two PSUM banks
(1024 wide) per scalar-engine ACTIVATE.

Softmax denominator comes from a ones column appended to V (attn@v
with M=65). The per-head normalization tail (reciprocal -> rank-1
broadcast matmul -> multiply) is deferred past the next head's score
matmuls so the in-order PE queue never stalls on the DVE reciprocal;
the broadcast lands in the unused partitions 64..127 of the same PSUM
bank as z.

All multi-MB DMAs are split into per-feature-tile chunks and issued
round-robin over the three DMA-capable queues (sync/scalar/gpsimd) —
a single dma_start rides one DMA engine at ~26 GB/s, so chunking is
what buys parallel HBM bandwidth.
"""

import math
import os
import sys

import numpy as np

for _p in ("/opt/trn_rl_repo", os.path.expanduser("~/.axon_site/_ro/trn_rl_repo")):
    if os.path.isdir(_p) and _p not in sys.path:
        sys.path.insert(0, _p)

import ml_dtypes  # noqa: E402

BF16 = ml_dtypes.bfloat16

DIM = 768
HEADS = 12
DK = 64
HID = 4 * DIM  # 3072
B, S = 2, 2048
N_CORES = 8
BLK = 512            # query tokens per core
NBLK = S // BLK      # 4 blocks per batch
EPS = 1e-5
ISCALE = 1.0 / math.sqrt(DIM)

FT = DIM // 128   # 6 feature tiles (== head pairs)
TT = BLK // 128   # 4 token tiles per core block
ST = S // 128     # 16 key token tiles per batch
HT = HID // 128   # 24 hidden tiles

_CACHE: dict = {}
TAPS = os.environ.get("KERNEL_TAPS", "0") == "1"


def _build_program():
    import concourse.bass as bass
    import concourse.mybir as mybir
    import concourse.tile as tile
    from concourse import bacc
    from concourse.masks import make_identity
    from concourse.tile import add_dep_helper

    f32 = mybir.dt.float32
    bf16 = mybir.dt.bfloat16
    AF = mybir.ActivationFunctionType
    ALU = mybir.AluOpType
    AX = mybir.AxisListType

    nc = bacc.Bacc()

    # ---- per-core DRAM I/O ----
    d_xT = nc.dram_tensor("xT", [DIM, S], bf16, kind="ExternalInput")
    d_xTb = nc.dram_tensor("xTb", [DIM, BLK], bf16, kind="ExternalInput")
    d_xb = nc.dram_tensor("xb", [BLK, DIM], f32, kind="ExternalInput")
    d_msk = nc.dram_tensor("msk", [S], f32, kind="ExternalInput")
    d_wq = nc.dram_tensor("wq", [DIM, DIM], bf16, kind="ExternalInput")
    d_wk = nc.dram_tensor("wk", [DIM, DIM], bf16, kind="ExternalInput")
    d_wv = nc.dram_tensor("wv", [DIM, DIM], bf16, kind="ExternalInput")
    d_wo = nc.dram_tensor("wo", [DIM, DIM], bf16, kind="ExternalInput")
    d_w1 = nc.dram_tensor("w1", [DIM, HID], bf16, kind="ExternalInput")
    d_w2 = nc.dram_tensor("w2", [HID, DIM], bf16, kind="ExternalInput")
    d_bq = nc.dram_tensor("bq", [DIM], f32, kind="ExternalInput")
    d_bk = nc.dram_tensor("bk", [DIM], f32, kind="ExternalInput")
    d_bv = nc.dram_tensor("bv", [DIM], f32, kind="ExternalInput")
    d_bo = nc.dram_tensor("bo", [DIM], f32, kind="ExternalInput")
    d_b1 = nc.dram_tensor("b1", [HID], f32, kind="ExternalInput")
    d_b2 = nc.dram_tensor("b2", [DIM], f32, kind="ExternalInput")
    d_g1 = nc.dram_tensor("g1", [DIM], f32, kind="ExternalInput")
    d_bb1 = nc.dram_tensor("bb1", [DIM], f32, kind="ExternalInput")
    d_g2 = nc.dram_tensor("g2", [DIM], f32, kind="ExternalInput")
    d_bb2 = nc.dram_tensor("bb2", [DIM], f32, kind="ExternalInput")
    d_out = nc.dram_tensor("out", [BLK, DIM], f32, kind="ExternalOutput")
    if TAPS:
        d_tap_zT = nc.dram_tensor("tap_zT", [128, FT, BLK], bf16, kind="ExternalOutput")
        d_tap_l1 = nc.dram_tensor("tap_l1", [128, TT, DIM], f32, kind="ExternalOutput")

    def bcast_ap(handle, n=128):
        ap = handle[:]
        return bass.AP(tensor=ap.tensor, offset=ap.offset, ap=[[0, n]] + list(ap.ap))

    with tile.TileContext(nc) as tc:
        with (
            tc.tile_pool(name="const", bufs=1) as const,
            tc.tile_pool(name="bigres", bufs=1) as big,
        ):
            # byte-balanced assignment over the 3 DMA queues (SP-HW, ACT-HW,
            # Pool-SW) — each queue drains sequentially at ~26GB/s, so both
            # balance and FIFO order matter
            _qload = [0, 0, 0]
            _qeng = [nc.sync, nc.scalar, nc.gpsimd]

            def q_dma(out, in_, nbytes):
                qi = min(range(3), key=lambda i: _qload[i])
                _qload[qi] += nbytes
                _qeng[qi].dma_start(out=out, in_=in_)

            # ---------- constants ----------
            # vector constants come in as tiny bf16 rows (gpsimd cast-DMA);
            # they are broadcast to 128 partitions on-chip via rank-1 matmuls
            # (1MB+ of stride-0 broadcast DMA would serialize the Pool queue)
            row_msk = const.tile([1, S], bf16)
            nc.gpsimd.dma_start(out=row_msk, in_=d_msk[:])
            row_bv = const.tile([1, DIM], bf16)
            nc.gpsimd.dma_start(out=row_bv, in_=d_bv[:])
            row_bo = const.tile([1, DIM], bf16)
            nc.gpsimd.dma_start(out=row_bo, in_=d_bo[:])
            row_b2 = const.tile([1, DIM], bf16)
            nc.gpsimd.dma_start(out=row_b2, in_=d_b2[:])
            row_g1 = const.tile([1, DIM], bf16)
            nc.gpsimd.dma_start(out=row_g1, in_=d_g1[:])
            row_bb1 = const.tile([1, DIM], bf16)
            nc.gpsimd.dma_start(out=row_bb1, in_=d_bb1[:])
            row_g2 = const.tile([1, DIM], bf16)
            nc.gpsimd.dma_start(out=row_g2, in_=d_g2[:])
            row_bb2 = const.tile([1, DIM], bf16)
            nc.gpsimd.dma_start(out=row_bb2, in_=d_bb2[:])
            sb_bq = const.tile([128, FT], f32)
            nc.sync.dma_start(out=sb_bq, in_=d_bq[:].rearrange("(t p) -> p t", p=128))
            sb_bk = const.tile([128, FT], f32)
            nc.sync.dma_start(out=sb_bk, in_=d_bk[:].rearrange("(t p) -> p t", p=128))
            sb_b1 = const.tile([128, HT], f32)
            nc.sync.dma_start(out=sb_b1, in_=d_b1[:].rearrange("(t p) -> p t", p=128))
            ident = const.tile([128, 128], f32)
            make_identity(nc, ident[:])
            ones64 = const.tile([1, 64], bf16)
            nc.vector.memset(ones64, 1.0)
            ones128 = const.tile([1, 128], bf16)
            nc.vector.memset(ones128, 1.0)
            eps_t = const.tile([128, 1], f32)
            nc.vector.memset(eps_t, EPS)

            mask_bc = const.tile([128, S], f32)
            bv_bc = const.tile([128, DIM], f32)
            bo_bc = const.tile([128, DIM], f32)
            b2_bc = const.tile([128, DIM], f32)
            g1_bc = const.tile([128, DIM], f32)
            bb1_bc = const.tile([128, DIM], f32)
            g2_bc = const.tile([128, DIM], f32)
            bb2_bc = const.tile([128, DIM], f32)

            def pe_bcast(dst, row, n, psum_tile_fn, chunk=512):
                # dst[128, n] = ones128^T @ row[1, n], by `chunk`-wide pieces
                for c0 in range(0, n, chunk):
                    w = min(chunk, n - c0)
                    pb = psum_tile_fn()
                    nc.tensor.matmul(
                        pb[:, 0:w], ones128[:], row[:, c0 : c0 + w],
                        start=True, stop=True,
                    )
                    nc.scalar.copy(dst[:, c0 : c0 + w], pb[:, 0:w])

            # persistent across attention->FFN boundary
            sb_l1 = big.tile([128, TT, DIM], f32)
            sb_l1T = big.tile([128, FT, BLK], bf16)

            # ---- attention residents (die after O-proj/LN1) ----
            attn_cm = tc.tile_pool(name="attn_res", bufs=1)
            attn_res = attn_cm.__enter__()
            sb_K = attn_res.tile([128, FT, S], bf16)    # K^T feat-major, masked
            sb_Q = attn_res.tile([128, FT, BLK], bf16)  # Q^T feat-major
            sb_V = attn_res.tile([128, ST, HEADS, DK + 1], bf16)  # V + ones col
            sb_zT = attn_res.tile([128, FT, BLK], bf16)  # z^T normalized

            wo_cm = tc.tile_pool(name="wo_p", bufs=1)
            wo_p = wo_cm.__enter__()

            # attention inner pools open first so the mask/bv on-chip
            # broadcasts land at the head of the ACT queue, ahead of the
            # weight-DMA issue instructions
            ets_cm = tc.tile_pool(name="ets", bufs=3)
            ets_p = ets_cm.__enter__()
            attsm_cm = tc.tile_pool(name="attsm", bufs=2)
            attsm = attsm_cm.__enter__()
            ps_qkv_cm = tc.tile_pool(name="ps_qkv", bufs=2, space="PSUM")
            ps_qkv = ps_qkv_cm.__enter__()
            ps_sc_cm = tc.tile_pool(name="ps_sc", bufs=2, space="PSUM")
            ps_sc = ps_sc_cm.__enter__()
            ps_z_cm = tc.tile_pool(name="ps_z", bufs=2, space="PSUM")
            ps_z = ps_z_cm.__enter__()

            nc.vector.memset(sb_V[:, :, :, DK : DK + 1], 1.0)

            def _sc_psum():
                t = ps_sc.tile([128, 2, 512], f32, tag="sc", name="bc_sc")
                return t[:, 0, :]

            pe_bcast(mask_bc, row_msk, S, _sc_psum)
            pe_bcast(bv_bc, row_bv, DIM, _sc_psum, chunk=384)

            # ---- QKV-phase residents, chunked per feature tile ----
            xt_cm = tc.tile_pool(name="xt_p", bufs=1)
            xt_p = xt_cm.__enter__()
            wqkv_cm = tc.tile_pool(name="wqkv_p", bufs=1)
            wqkv_p = wqkv_cm.__enter__()

            # wk + xT first halves feed the first K chains — balance them
            # across all 3 queues ahead of everything else
            wk_t, xt_t = [], []
            for kt in range(FT):
                wkt = wqkv_p.tile([128, DIM], bf16, tag=f"wk{kt}")
                q_dma(wkt, d_wk[kt * 128 : (kt + 1) * 128, :], DIM * 128 * 2)
                wk_t.append(wkt)
                # both halves on one queue: cross-queue writes to the same
                # tile serialize on a conservative WAW dep anyway
                xtt = xt_p.tile([128, S], bf16, tag=f"xt{kt}")
                qi = min(range(3), key=lambda i: _qload[i])
                for hf in (0, 1):
                    _qload[qi] += 1024 * 128 * 2
                    _qeng[qi].dma_start(
                        out=xtt[:, hf * 1024 : (hf + 1) * 1024],
                        in_=d_xT[kt * 128 : (kt + 1) * 128,
                                 hf * 1024 : (hf + 1) * 1024],
                    )
                xt_t.append(xtt)
            xtb_t, wq_t = [], []
            for kt in range(FT):
                xtbt = xt_p.tile([128, BLK], bf16, tag=f"xtb{kt}")
                q_dma(xtbt, d_xTb[kt * 128 : (kt + 1) * 128, :], BLK * 128 * 2)
                xtb_t.append(xtbt)
                wqt = wqkv_p.tile([128, DIM], bf16, tag=f"wq{kt}")
                q_dma(wqt, d_wq[kt * 128 : (kt + 1) * 128, :], DIM * 128 * 2)
                wq_t.append(wqt)
            wv_t = []
            for kt in range(FT):
                wvt = wqkv_p.tile([128, DIM], bf16, tag=f"wv{kt}")
                q_dma(wvt, d_wv[kt * 128 : (kt + 1) * 128, :], DIM * 128 * 2)
                wv_t.append(wvt)
            wo_t = []
            for kt in range(FT):
                wot = wo_p.tile([128, DIM], bf16, tag=f"wo{kt}")
                q_dma(wot, d_wo[kt * 128 : (kt + 1) * 128, :], DIM * 128 * 2)
                wo_t.append(wot)

            # ============ QKV + attention, streamed per head pair ============
            # deferred normalization tails, two-stage: the reciprocal is
            # emitted at the start of the NEXT half (so it sits behind that
            # ft's K-copy ops in the DVE queue instead of ahead of them),
            # and the rank-1 broadcast + multiply at that half's b8==5
            pending: list = []

            def emit_recip():
                if pending and pending[0].get("rsum_bf") is None:
                    e = pending[0]
                    rsum = attsm.tile([1, BLK], f32, tag="rsum")
                    nc.vector.reciprocal(rsum, e["zp"][DK : DK + 1, :])
                    rsum_bf = attsm.tile([1, BLK], bf16, tag="rsbf")
                    nc.vector.tensor_copy(rsum_bf, rsum)
                    e["rsum_bf"] = rsum_bf

            def flush_tail(anchor=None):
                if not pending:
                    return
                e = pending.pop(0)
                zp = e["zp"]
                rbp = zp[DK : DK + DK, :]
                rbmm = nc.tensor.matmul(
                    rbp, ones64[:], e["rsum_bf"], start=True, stop=True
                )
                if anchor is not None:
                    # pin the broadcast matmul behind the current score matmul
                    # so the list scheduler cannot hoist it ahead into a spot
                    # where the PE queue stalls on the DVE reciprocal
                    add_dep_helper(
                        rbmm.ins, anchor.ins, sync=False,
                        reason="defer rb matmul behind scores",
                    )
                rb = attsm.tile([64, BLK], f32, tag="rbs")
                nc.vector.tensor_copy(rb, rbp)
                nc.vector.tensor_mul(
                    sb_zT[e["ho"] : e["ho"] + 64, e["ft"], :], zp[0:DK, :], rb
                )

            for ft in range(FT):
                # K^T[ft] over the whole batch, bias + mask folded in
                for nt in range(S // 512):
                    ps = ps_qkv.tile([128, 512], f32, tag="p")
                    for kt in range(FT):
                        nc.tensor.matmul(
                            ps,
                            wk_t[kt][:, ft * 128 : (ft + 1) * 128],
                            xt_t[kt][:, nt * 512 : (nt + 1) * 512],
                            start=(kt == 0),
                            stop=(kt == FT - 1),
                        )
                    nc.vector.scalar_tensor_tensor(
                        out=sb_K[:, ft, nt * 512 : (nt + 1) * 512],
                        in0=ps,
                        scalar=sb_bk[:, ft : ft + 1],
                        in1=mask_bc[:, nt * 512 : (nt + 1) * 512],
                        op0=ALU.add,
                        op1=ALU.mult,
                    )
                # Q^T[ft] for the core's own block
                ps = ps_qkv.tile([128, 512], f32, tag="p")
                for kt in range(FT):
                    nc.tensor.matmul(
                        ps,
                        wq_t[kt][:, ft * 128 : (ft + 1) * 128],
                        xtb_t[kt],
                        start=(kt == 0),
                        stop=(kt == FT - 1),
                    )
                nc.vector.tensor_scalar_add(sb_Q[:, ft, :], ps, sb_bq[:, ft : ft + 1])

                if ft == 0:
                    # V tok-major over the whole batch, laid out [tok, head, dk+1].
                    # Must be complete before the first z matmul below.
                    for nh in range(2):
                        for tt2 in range(ST):
                            psv = ps_qkv.tile([128, 512], f32, tag="p")
                            for kt in range(FT):
                                nc.tensor.matmul(
                                    psv[:, 0:384],
                                    xt_t[kt][:, tt2 * 128 : (tt2 + 1) * 128],
                                    wv_t[kt][:, nh * 384 : (nh + 1) * 384],
                                    start=(kt == 0),
                                    stop=(kt == FT - 1),
                                )
                            nc.vector.scalar_tensor_tensor(
                                out=sb_V[:, tt2, nh * 6 : (nh + 1) * 6, 0:DK],
                                in0=psv[:, 0:384].rearrange("p (h d) -> p h d", d=DK),
                                scalar=1.0,
                                in1=bv_bc[:, nh * 384 : (nh + 1) * 384].rearrange(
                                    "p (h d) -> p h d", d=DK
                                ),
                                op0=ALU.mult,
                                op1=ALU.add,
                            )

                # scores + exp + z for the two heads of this feature tile
                for half in (0, 1):
                    h = 2 * ft + half
                    ho = half * 64
                    zp = ps_z.tile([128, BLK], f32, tag="z")
                    emit_recip()
                    for b8 in range(ST // 2):
                        pssc = ps_sc.tile([128, 2, 512], f32, tag="sc")
                        last_sc = None
                        for j in (0, 1):
                            kt2 = b8 * 2 + j
                            last_sc = nc.tensor.matmul(
                                pssc[:, j, :],
                                sb_K[ho : ho + 64, ft, kt2 * 128 : (kt2 + 1) * 128],
                                sb_Q[ho : ho + 64, ft, :],
                                start=True,
                                stop=True,
                            )
                        et = ets_p.tile([128, 2, BLK], bf16, tag="exp")
                        nc.scalar.activation(
                            et[:].rearrange("p a b -> p (a b)"),
                            pssc[:].rearrange("p a b -> p (a b)"),
                            AF.Exp,
                            scale=ISCALE,
                        )
                        for j in (0, 1):
                            kt2 = b8 * 2 + j
                            nc.tensor.matmul(
                                zp[0 : DK + 1, :],
                                sb_V[:, kt2, h, :],
                                et[:, j, :],
                                start=(kt2 == 0),
                                stop=(kt2 == ST - 1),
                            )
                        if b8 == 5:
                            # previous head's normalization tail goes here so
                            # the PE never waits on the DVE reciprocal (the
                            # rank-1 matmul has a WAR on the denominator row
                            # the reciprocal reads; ~5us of scores gives the
                            # ~2.5us reciprocal time to drain)
                            flush_tail(anchor=last_sc)
                    pending.append({"zp": zp, "ho": ho, "ft": ft})

            emit_recip()
            flush_tail()

            if TAPS:
                nc.sync.dma_start(out=d_tap_zT[:], in_=sb_zT)

            wqkv_cm.__exit__(None, None, None)
            xt_cm.__exit__(None, None, None)
            ps_z_cm.__exit__(None, None, None)
            ps_sc_cm.__exit__(None, None, None)
            ps_qkv_cm.__exit__(None, None, None)
            attsm_cm.__exit__(None, None, None)
            ets_cm.__exit__(None, None, None)

            # ============ O proj + LN1 (+residual) + l1 transpose ============
            def layer_norm_to(out_ap, x_ap, g_bc_t, resid_ap, pool):
                # spread across Pool (reduce/sub, SBUF-only) / ACT (sq, sqrt)
                # / DVE (recip, fused scale+gain, final add)
                s = pool.tile([128, 1], f32, tag="ln_s")
                nc.vector.tensor_reduce(s, x_ap, axis=AX.X, op=ALU.add)
                mean = pool.tile([128, 1], f32, tag="ln_m")
                nc.vector.tensor_scalar_mul(mean, s, 1.0 / DIM)
                xc = pool.tile([128, DIM], f32, tag="ln_xc")
                nc.vector.tensor_scalar(xc, x_ap, mean, None, op0=ALU.subtract)
                junk = pool.tile([128, DIM], bf16, tag="ln_j")
                var = pool.tile([128, 1], f32, tag="ln_v")
                nc.scalar.activation(junk, xc, AF.Square, accum_out=var)
                sd = pool.tile([128, 1], f32, tag="ln_sd")
                nc.scalar.activation(sd, var, AF.Sqrt, bias=eps_t[:], scale=1.0 / DIM)
                rstd = pool.tile([128, 1], f32, tag="ln_r")
                nc.vector.reciprocal(rstd, sd)
                tg = pool.tile([128, DIM], f32, tag="ln_tg")
                nc.vector.scalar_tensor_tensor(
                    out=tg, in0=xc, scalar=rstd, in1=g_bc_t,
                    op0=ALU.mult, op1=ALU.mult,
                )
                nc.vector.tensor_add(out_ap, tg, resid_ap)

            xb_r = d_xb[:].rearrange("(t p) d -> p t d", p=128)
            with (
                tc.tile_pool(name="ln1p", bufs=2) as ln1p,
                tc.tile_pool(name="ps_o", bufs=2, space="PSUM") as ps_o,
                tc.tile_pool(name="ps_t", bufs=2, space="PSUM") as ps_t,
            ):
                def _o_psum():
                    t = ps_o.tile([128, 384], f32, tag="op", name="bc_o")
                    return t[:]

                pe_bcast(bo_bc, row_bo, DIM, _o_psum, chunk=384)
                pe_bcast(g1_bc, row_g1, DIM, _o_psum, chunk=384)
                pe_bcast(bb1_bc, row_bb1, DIM, _o_psum, chunk=384)
                for tt in range(TT):
                    l1pre = ln1p.tile([128, DIM], f32, tag="l1pre")
                    for nh in range(2):
                        ps = ps_o.tile([128, 384], f32, tag="op")
                        for kt in range(FT):
                            nc.tensor.matmul(
                                ps,
                                sb_zT[:, kt, tt * 128 : (tt + 1) * 128],
                                wo_t[kt][:, nh * 384 : (nh + 1) * 384],
                                start=(kt == 0),
                                stop=(kt == FT - 1),
                            )
                        nc.vector.scalar_tensor_tensor(
                            out=l1pre[:, nh * 384 : (nh + 1) * 384],
                            in0=ps,
                            scalar=1.0,
                            in1=bo_bc[:, nh * 384 : (nh + 1) * 384],
                            op0=ALU.mult,
                            op1=ALU.add,
                        )
                    xbt = ln1p.tile([128, DIM], f32, tag="xbt")
                    nc.sync.dma_start(out=xbt, in_=xb_r[:, tt, :])
                    xb1 = ln1p.tile([128, DIM], f32, tag="xb1")
                    nc.vector.tensor_add(xb1, xbt, bb1_bc)
                    layer_norm_to(sb_l1[:, tt, :], l1pre[:], g1_bc, xb1, ln1p)
                    # transpose l1[tt] right away so FFN1 can start early;
                    # PSUM->SBUF copies on ACT (idle after attention)
                    for ft in range(FT):
                        pst = ps_t.tile([128, 128], f32, tag="tp")
                        nc.tensor.transpose(
                            pst, sb_l1[:, tt, ft * 128 : (ft + 1) * 128], ident[:]
                        )
                        nc.scalar.copy(
                            sb_l1T[:, ft, tt * 128 : (tt + 1) * 128], pst
                        )

            if TAPS:
                nc.sync.dma_start(out=d_tap_l1[:], in_=sb_l1)

            wo_cm.__exit__(None, None, None)
            attn_cm.__exit__(None, None, None)

            # ============ FFN1 -> hT, FFN2 streamed behind it ============
            w1_cm = tc.tile_pool(name="w1_p", bufs=1)
            w1_p = w1_cm.__enter__()
            w1_t = []
            for kt in range(FT):
                w1t = w1_p.tile([128, HID], bf16, tag=f"w1_{kt}")
                q_dma(w1t, d_w1[kt * 128 : (kt + 1) * 128, :], HID * 128 * 2)
                w1_t.append(w1t)
            hT_cm = tc.tile_pool(name="hT_p", bufs=1)
            hT_p = hT_cm.__enter__()
            sb_hT = hT_p.tile([128, HT, BLK], bf16)  # relu(ffn1)^T, hid-major

            with (
                tc.tile_pool(name="w2s", bufs=1) as w2s_p,
                tc.tile_pool(name="ln2p", bufs=2) as ln2p,
                tc.tile_pool(name="f2pre_p", bufs=4) as f2pre_p,
                tc.tile_pool(name="outp", bufs=3) as outp,
                tc.tile_pool(name="ps_f1", bufs=2, space="PSUM") as ps_f1,
                tc.tile_pool(name="ps_f2", bufs=4, space="PSUM") as ps_f2,
            ):
                # prefetch all of w2 in 8 chunks (4 kt-groups x 2 halves)
                w2c = {}
                for nh in range(2):
                    for kg in range(4):
                        w2t = w2s_p.tile([128, 6, 384], bf16, tag=f"w2c{nh}{kg}")
                        q_dma(
                            w2t,
                            d_w2[
                                kg * 768 : (kg + 1) * 768,
                                nh * 384 : (nh + 1) * 384,
                            ].rearrange("(t p) c -> p t c", p=128),
                            6 * 384 * 128 * 2,
                        )
                        w2c[(nh, kg)] = w2t

                out_r = d_out[:].rearrange("(t p) d -> p t d", p=128)

                def _f2_psum():
                    t = ps_f2.tile([128, 384], f32, tag="f2", name="bc_f2")
                    return t[:]

                pe_bcast(b2_bc, row_b2, DIM, _f2_psum, chunk=384)
                pe_bcast(g2_bc, row_g2, DIM, _f2_psum, chunk=384)
                pe_bcast(bb2_bc, row_bb2, DIM, _f2_psum, chunk=384)

                def ffn1_half(hf):
                    # FFN1 for token half hf (cols hf*256 .. hf*256+256)
                    for ht2 in range(HT):
                        ps = ps_f1.tile([128, 256], f32, tag="f1")
                        for kt in range(FT):
                            nc.tensor.matmul(
                                ps,
                                w1_t[kt][:, ht2 * 128 : (ht2 + 1) * 128],
                                sb_l1T[:, kt, hf * 256 : (hf + 1) * 256],
                                start=(kt == 0),
                                stop=(kt == FT - 1),
                            )
                        # relu(x + b1) on DVE: (x add b1) max 0
                        nc.vector.tensor_scalar(
                            sb_hT[:, ht2, hf * 256 : (hf + 1) * 256], ps,
                            sb_b1[:, ht2 : ht2 + 1], 0.0,
                            op0=ALU.add, op1=ALU.max,
                        )

                def ffn2_tile(tt):
                    # both 384-col halves for one token tile, then LN2 + out
                    f2pre = f2pre_p.tile([128, DIM], f32, tag="f2pre")
                    for nh in range(2):
                        chain = ps_f2.tile([128, 384], f32, tag="f2")
                        for kt in range(HT):
                            nc.tensor.matmul(
                                chain,
                                sb_hT[:, kt, tt * 128 : (tt + 1) * 128],
                                w2c[(nh, kt // 6)][:, kt % 6, :],
                                start=(kt == 0),
                                stop=(kt == HT - 1),
                            )
                        nc.vector.scalar_tensor_tensor(
                            out=f2pre[:, nh * 384 : (nh + 1) * 384],
                            in0=chain,
                            scalar=1.0,
                            in1=b2_bc[:, nh * 384 : (nh + 1) * 384],
                            op0=ALU.mult,
                            op1=ALU.add,
                        )
                    l1b = ln2p.tile([128, DIM], f32, tag="l1b")
                    nc.vector.tensor_add(l1b, sb_l1[:, tt, :], bb2_bc)
                    o_sb = outp.tile([128, DIM], f32, tag="osb")
                    layer_norm_to(o_sb[:], f2pre[:], g2_bc, l1b, ln2p)
                    nc.sync.dma_start(out=out_r[:, tt, :], in_=o_sb)

                ffn1_half(0)
                ffn2_tile(0)
                ffn2_tile(1)
                ffn1_half(1)
                ffn2_tile(2)
                ffn2_tile(3)

            hT_cm.__exit__(None, None, None)
            w1_cm.__exit__(None, None, None)

    return nc


def _get_nc(finalized=True):
    if "nc" not in _CACHE:
        _CACHE["nc"] = _build_program()
    nc = _CACHE["nc"]
    if finalized and not nc.is_finalized():
        nc.finalize()
    return nc


def make_in_maps(inputs: dict) -> list:
    x = np.asarray(inputs["x_n"], np.float32).reshape(B, S, DIM)
    mask = np.asarray(inputs["mask"]).reshape(B, S)
    w = {
        k: np.ascontiguousarray(np.asarray(inputs[k], np.float32).astype(BF16))
        for k in ("wq", "wk", "wv", "wo", "w1", "w2")
    }
    vecs = {
        "bq": inputs["bq"], "bk": inputs["bk"], "bv": inputs["bv"],
        "bo": inputs["bo"], "b1": inputs["b1"], "b2": inputs["b2"],
        "g1": inputs["ln1_g"], "bb1": inputs["ln1_b"],
        "g2": inputs["ln2_g"], "bb2": inputs["ln2_b"],
    }
    vecs = {k: np.ascontiguousarray(np.asarray(v, np.float32)) for k, v in vecs.items()}
    in_maps = []
    for c in range(N_CORES):
        b, blk = c // NBLK, c % NBLK
        xb_full = x[b]
        xT = np.ascontiguousarray(xb_full.T.astype(BF16))
        xblk = np.ascontiguousarray(xb_full[blk * BLK : (blk + 1) * BLK])
        xTb = np.ascontiguousarray(xblk.T.astype(BF16))
        msk = (mask[b] != 0).astype(np.float32)
        m = {"xT": xT, "xTb": xTb, "xb": xblk, "msk": msk}
        m.update(w)
        m.update(vecs)
        in_maps.append(m)
    return in_maps


def assemble(per_core_out: list) -> np.ndarray:
    blocks = [np.asarray(o, np.float32) for o in per_core_out]
    full = np.concatenate(blocks, axis=0).reshape(B, S, DIM)
    return full


def kernel(**inputs) -> np.ndarray:
    from concourse.bass_utils import run_bass_kernel_spmd

    nc = _get_nc()
    in_maps = make_in_maps(inputs)
    res = run_bass_kernel_spmd(nc, in_maps, list(range(N_CORES)))
    return assemble([r["out"] for r in res.results])


# revision 50
# speedup vs baseline: 1.0499x; 1.0499x over previous
"""Trainium2 Bass kernel for a dense transformer encoder layer.

Model (faithful to the oracle):
  q,k,v = x@wq+bq, x@wk+bk, x@wv+bv          (12 heads, dk=64, DIM=768)
  scores = q@k^T / sqrt(768)  (note: sqrt(dim_model), not sqrt(dk))
  scores[mask==0] = 1e-11  (NOT -inf; masked keys still contribute ~1/Z)
  attn = softmax(scores); z = attn@v; o = z@wo+bo
  l1 = x + LN(o);  ffn = relu(l1@w1+b1)@w2+b2;  out = l1 + LN(ffn)

Sharding: 4096 tokens (B=2,S=2048) split 8 ways -> 512 query tokens per
core. Cores 0-3 own batch 0, cores 4-7 batch 1. K/V are computed for
the core's whole batch (redundantly within each 4-core group) so there
are NO collectives: cores run fully independently, immune to cross-core
dispatch skew.

Mask trick: the key mask is folded into K at projection time:
K_masked[:,kpos] = (K[:,kpos]+bk) * m[kpos], m in {0,1}. Masked keys
produce scores == 0 exactly and exp(0) = 1.0 == fp32(exp(1e-11)),
matching the oracle bit-for-bit in fp32. The exp scale is then a
compile-time constant, so score tiles are exp'ed two PSUM banks
(1024 wide) per scalar-engine ACTIVATE.

Softmax denominator comes from a ones column appended to V (attn@v
with M=65). The per-head normalization tail (reciprocal -> rank-1
broadcast matmul -> multiply) is deferred past the next head's score
matmuls so the in-order PE queue never stalls on the DVE reciprocal;
the broadcast lands in the unused partitions 64..127 of the same PSUM
bank as z.

All multi-MB DMAs are split into per-feature-tile chunks and issued
round-robin over the three DMA-capable queues (sync/scalar/gpsimd) —
a single dma_start rides one DMA engine at ~26 GB/s, so chunking is
what buys parallel HBM bandwidth.
"""

import math
import os
import sys

import numpy as np

for _p in ("/opt/trn_rl_repo", os.path.expanduser("~/.axon_site/_ro/trn_rl_repo")):
    if os.path.isdir(_p) and _p not in sys.path:
        sys.path.insert(0, _p)

import ml_dtypes  # noqa: E402

BF16 = ml_dtypes.bfloat16

DIM = 768
HEADS = 12
DK = 64
HID = 4 * DIM  # 3072
B, S = 2, 2048
N_CORES = 8
BLK = 512            # query tokens per core
NBLK = S // BLK      # 4 blocks per batch
EPS = 1e-5
ISCALE = 1.0 / math.sqrt(DIM)

FT = DIM // 128   # 6 feature tiles (== head pairs)
TT = BLK // 128   # 4 token tiles per core block
ST = S // 128     # 16 key token tiles per batch
HT = HID // 128   # 24 hidden tiles

_CACHE: dict = {}
TAPS = os.environ.get("KERNEL_TAPS", "0") == "1"


def _build_program():
    import concourse.bass as bass
    import concourse.mybir as mybir
    import concourse.tile as tile
    from concourse import bacc
    from concourse.masks import make_identity
    from concourse.tile import add_dep_helper

    f32 = mybir.dt.float32
    bf16 = mybir.dt.bfloat16
    AF = mybir.ActivationFunctionType
    ALU = mybir.AluOpType
    AX = mybir.AxisListType

    nc = bacc.Bacc()

    # ---- per-core DRAM I/O ----
    d_xT = nc.dram_tensor("xT", [DIM, S], bf16, kind="ExternalInput")
    d_xTb = nc.dram_tensor("xTb", [DIM, BLK], bf16, kind="ExternalInput")
    d_xb = nc.dram_tensor("xb", [BLK, DIM], f32, kind="ExternalInput")
    d_msk = nc.dram_tensor("msk", [S], f32, kind="ExternalInput")
    d_wq = nc.dram_tensor("wq", [DIM, DIM], bf16, kind="ExternalInput")
    d_wk = nc.dram_tensor("wk", [DIM, DIM], bf16, kind="ExternalInput")
    d_wv = nc.dram_tensor("wv", [DIM, DIM], bf16, kind="ExternalInput")
    d_wo = nc.dram_tensor("wo", [DIM, DIM], bf16, kind="ExternalInput")
    d_w1 = nc.dram_tensor("w1", [DIM, HID], bf16, kind="ExternalInput")
    d_w2 = nc.dram_tensor("w2", [HID, DIM], bf16, kind="ExternalInput")
    d_bq = nc.dram_tensor("bq", [DIM], f32, kind="ExternalInput")
    d_bk = nc.dram_tensor("bk", [DIM], f32, kind="ExternalInput")
    d_bv = nc.dram_tensor("bv", [DIM], f32, kind="ExternalInput")
    d_bo = nc.dram_tensor("bo", [DIM], f32, kind="ExternalInput")
    d_b1 = nc.dram_tensor("b1", [HID], f32, kind="ExternalInput")
    d_b2 = nc.dram_tensor("b2", [DIM], f32, kind="ExternalInput")
    d_g1 = nc.dram_tensor("g1", [DIM], f32, kind="ExternalInput")
    d_bb1 = nc.dram_tensor("bb1", [DIM], f32, kind="ExternalInput")
    d_g2 = nc.dram_tensor("g2", [DIM], f32, kind="ExternalInput")
    d_bb2 = nc.dram_tensor("bb2", [DIM], f32, kind="ExternalInput")
    d_out = nc.dram_tensor("out", [BLK, DIM], f32, kind="ExternalOutput")
    if TAPS:
        d_tap_zT = nc.dram_tensor("tap_zT", [128, FT, BLK], bf16, kind="ExternalOutput")
        d_tap_l1 = nc.dram_tensor("tap_l1", [128, TT, DIM], f32, kind="ExternalOutput")

    def bcast_ap(handle, n=128):
        ap = handle[:]
        return bass.AP(tensor=ap.tensor, offset=ap.offset, ap=[[0, n]] + list(ap.ap))

    with tile.TileContext(nc) as tc:
        with (
            tc.tile_pool(name="const", bufs=1) as const,
            tc.tile_pool(name="bigres", bufs=1) as big,
        ):
            # byte-balanced assignment over the 3 DMA queues (SP-HW, ACT-HW,
            # Pool-SW) — each queue drains sequentially at ~26GB/s, so both
            # balance and FIFO order matter
            _qload = [0, 0, 0]
            _qeng = [nc.sync, nc.scalar, nc.gpsimd]

            def q_dma(out, in_, nbytes):
                qi = min(range(3), key=lambda i: _qload[i])
                _qload[qi] += nbytes
                _qeng[qi].dma_start(out=out, in_=in_)

            # ---------- constants ----------
            # vector constants come in as tiny bf16 rows (gpsimd cast-DMA);
            # they are broadcast to 128 partitions on-chip via rank-1 matmuls
            # (1MB+ of stride-0 broadcast DMA would serialize the Pool queue)
            row_msk = const.tile([1, S], bf16)
            nc.gpsimd.dma_start(out=row_msk, in_=d_msk[:])
            row_bv = const.tile([1, DIM], bf16)
            nc.gpsimd.dma_start(out=row_bv, in_=d_bv[:])
            row_bo = const.tile([1, DIM], bf16)
            nc.gpsimd.dma_start(out=row_bo, in_=d_bo[:])
            row_b2 = const.tile([1, DIM], bf16)
            nc.gpsimd.dma_start(out=row_b2, in_=d_b2[:])
            row_g1 = const.tile([1, DIM], bf16)
            nc.gpsimd.dma_start(out=row_g1, in_=d_g1[:])
            row_bb1 = const.tile([1, DIM], bf16)
            nc.gpsimd.dma_start(out=row_bb1, in_=d_bb1[:])
            row_g2 = const.tile([1, DIM], bf16)
            nc.gpsimd.dma_start(out=row_g2, in_=d_g2[:])
            row_bb2 = const.tile([1, DIM], bf16)
            nc.gpsimd.dma_start(out=row_bb2, in_=d_bb2[:])
            sb_bq = const.tile([128, FT], f32)
            nc.sync.dma_start(out=sb_bq, in_=d_bq[:].rearrange("(t p) -> p t", p=128))
            sb_bk = const.tile([128, FT], f32)
            nc.sync.dma_start(out=sb_bk, in_=d_bk[:].rearrange("(t p) -> p t", p=128))
            sb_b1 = const.tile([128, HT], f32)
            nc.sync.dma_start(out=sb_b1, in_=d_b1[:].rearrange("(t p) -> p t", p=128))
            ident = const.tile([128, 128], f32)
            make_identity(nc, ident[:])
            ones64 = const.tile([1, 64], bf16)
            nc.vector.memset(ones64, 1.0)
            ones128 = const.tile([1, 128], bf16)
            nc.vector.memset(ones128, 1.0)
            eps_t = const.tile([128, 1], f32)
            nc.vector.memset(eps_t, EPS)

            mask_bc = const.tile([128, S], f32)
            bv_bc = const.tile([128, DIM], f32)
            bo_bc = const.tile([128, DIM], f32)
            b2_bc = const.tile([128, DIM], f32)
            g1_bc = const.tile([128, DIM], f32)
            bb1_bc = const.tile([128, DIM], f32)
            g2_bc = const.tile([128, DIM], f32)
            bb2_bc = const.tile([128, DIM], f32)

            def pe_bcast(dst, row, n, psum_tile_fn, chunk=512):
                # dst[128, n] = ones128^T @ row[1, n], by `chunk`-wide pieces
                for c0 in range(0, n, chunk):
                    w = min(chunk, n - c0)
                    pb = psum_tile_fn()
                    nc.tensor.matmul(
                        pb[:, 0:w], ones128[:], row[:, c0 : c0 + w],
                        start=True, stop=True,
                    )
                    nc.scalar.copy(dst[:, c0 : c0 + w], pb[:, 0:w])

            # persistent across attention->FFN boundary
            sb_l1 = big.tile([128, TT, DIM], f32)
            sb_l1T = big.tile([128, FT, BLK], bf16)

            # ---- attention residents (die after O-proj/LN1) ----
            attn_cm = tc.tile_pool(name="attn_res", bufs=1)
            attn_res = attn_cm.__enter__()
            sb_K = attn_res.tile([128, FT, S], bf16)    # K^T feat-major, masked
            sb_Q = attn_res.tile([128, FT, BLK], bf16)  # Q^T feat-major
            sb_V = attn_res.tile([128, ST, HEADS, DK + 1], bf16)  # V + ones col
            sb_zT = attn_res.tile([128, FT, BLK], bf16)  # z^T normalized

            wo_cm = tc.tile_pool(name="wo_p", bufs=1)
            wo_p = wo_cm.__enter__()

            # attention inner pools open first so the mask/bv on-chip
            # broadcasts land at the head of the ACT queue, ahead of the
            # weight-DMA issue instructions
            ets_cm = tc.tile_pool(name="ets", bufs=3)
            ets_p = ets_cm.__enter__()
            attsm_cm = tc.tile_pool(name="attsm", bufs=2)
            attsm = attsm_cm.__enter__()
            ps_qkv_cm = tc.tile_pool(name="ps_qkv", bufs=2, space="PSUM")
            ps_qkv = ps_qkv_cm.__enter__()
            ps_sc_cm = tc.tile_pool(name="ps_sc", bufs=2, space="PSUM")
            ps_sc = ps_sc_cm.__enter__()
            ps_z_cm = tc.tile_pool(name="ps_z", bufs=2, space="PSUM")
            ps_z = ps_z_cm.__enter__()

            nc.vector.memset(sb_V[:, :, :, DK : DK + 1], 1.0)

            def _sc_psum():
                t = ps_sc.tile([128, 2, 512], f32, tag="sc", name="bc_sc")
                return t[:, 0, :]

            pe_bcast(mask_bc, row_msk, S, _sc_psum)
            pe_bcast(bv_bc, row_bv, DIM, _sc_psum, chunk=384)

            # ---- QKV-phase residents, chunked per feature tile ----
            xt_cm = tc.tile_pool(name="xt_p", bufs=1)
            xt_p = xt_cm.__enter__()
            wqkv_cm = tc.tile_pool(name="wqkv_p", bufs=1)
            wqkv_p = wqkv_cm.__enter__()

            # wk + xT first halves feed the first K chains — balance them
            # across all 3 queues ahead of everything else
            wk_t, xt_t = [], []
            for kt in range(FT):
                wkt = wqkv_p.tile([128, DIM], bf16, tag=f"wk{kt}")
                q_dma(wkt, d_wk[kt * 128 : (kt + 1) * 128, :], DIM * 128 * 2)
                wk_t.append(wkt)
                # both halves on one queue: cross-queue writes to the same
                # tile serialize on a conservative WAW dep anyway
                xtt = xt_p.tile([128, S], bf16, tag=f"xt{kt}")
                qi = min(range(3), key=lambda i: _qload[i])
                for hf in (0, 1):
                    _qload[qi] += 1024 * 128 * 2
                    _qeng[qi].dma_start(
                        out=xtt[:, hf * 1024 : (hf + 1) * 1024],
                        in_=d_xT[kt * 128 : (kt + 1) * 128,
                                 hf * 1024 : (hf + 1) * 1024],
                    )
                xt_t.append(xtt)
            xtb_t, wq_t = [], []
            for kt in range(FT):
                xtbt = xt_p.tile([128, BLK], bf16, tag=f"xtb{kt}")
                q_dma(xtbt, d_xTb[kt * 128 : (kt + 1) * 128, :], BLK * 128 * 2)
                xtb_t.append(xtbt)
                wqt = wqkv_p.tile([128, DIM], bf16, tag=f"wq{kt}")
                q_dma(wqt, d_wq[kt * 128 : (kt + 1) * 128, :], DIM * 128 * 2)
                wq_t.append(wqt)
            wv_t = []
            for kt in range(FT):
                wvt = wqkv_p.tile([128, DIM], bf16, tag=f"wv{kt}")
                q_dma(wvt, d_wv[kt * 128 : (kt + 1) * 128, :], DIM * 128 * 2)
                wv_t.append(wvt)
            wo_t = []
            for kt in range(FT):
                wot = wo_p.tile([128, DIM], bf16, tag=f"wo{kt}")
                q_dma(wot, d_wo[kt * 128 : (kt + 1) * 128, :], DIM * 128 * 2)
                wo_t.append(wot)

            # ============ QKV + attention, streamed per head pair ============
            # deferred normalization tails, two-stage: the reciprocal is
            # emitted at the start of the NEXT half (so it sits behind that
            # ft's K-copy ops in the DVE queue instead of ahead of them),
            # and the rank-1 broadcast + multiply at that half's b8==5
            pending: list = []

            def emit_recip():
                if pending and pending[0].get("rsum_bf") is None:
                    e = pending[0]
                    rsum = attsm.tile([1, BLK], f32, tag="rsum")
                    nc.vector.reciprocal(rsum, e["zp"][DK : DK + 1, :])
                    rsum_bf = attsm.tile([1, BLK], bf16, tag="rsbf")
                    nc.vector.tensor_copy(rsum_bf, rsum)
                    e["rsum_bf"] = rsum_bf

            def flush_tail(anchor=None):
                if not pending:
                    return
                e = pending.pop(0)
                zp = e["zp"]
                rbp = zp[DK : DK + DK, :]
                rbmm = nc.tensor.matmul(
                    rbp, ones64[:], e["rsum_bf"], start=True, stop=True
                )
                if anchor is not None:
                    # pin the broadcast matmul behind the current score matmul
                    # so the list scheduler cannot hoist it ahead into a spot
                    # where the PE queue stalls on the DVE reciprocal
                    add_dep_helper(
                        rbmm.ins, anchor.ins, sync=False,
                        reason="defer rb matmul behind scores",
                    )
                rb = attsm.tile([64, BLK], f32, tag="rbs")
                nc.vector.tensor_copy(rb, rbp)
                nc.vector.tensor_mul(
                    sb_zT[e["ho"] : e["ho"] + 64, e["ft"], :], zp[0:DK, :], rb
                )

            # projection chain emitters (6 accumulating matmuls + 1 DVE op)
            def emit_K(ftn, nt):
                ps = ps_qkv.tile([128, 512], f32, tag="p", name="kq_ps")
                for kt in range(FT):
                    nc.tensor.matmul(
                        ps,
                        wk_t[kt][:, ftn * 128 : (ftn + 1) * 128],
                        xt_t[kt][:, nt * 512 : (nt + 1) * 512],
                        start=(kt == 0),
                        stop=(kt == FT - 1),
                    )
                nc.vector.scalar_tensor_tensor(
                    out=sb_K[:, ftn, nt * 512 : (nt + 1) * 512],
                    in0=ps,
                    scalar=sb_bk[:, ftn : ftn + 1],
                    in1=mask_bc[:, nt * 512 : (nt + 1) * 512],
                    op0=ALU.add,
                    op1=ALU.mult,
                )

            def emit_Q(ftn):
                ps = ps_qkv.tile([128, 512], f32, tag="p", name="kq_ps")
                for kt in range(FT):
                    nc.tensor.matmul(
                        ps,
                        wq_t[kt][:, ftn * 128 : (ftn + 1) * 128],
                        xtb_t[kt],
                        start=(kt == 0),
                        stop=(kt == FT - 1),
                    )
                nc.vector.tensor_scalar_add(
                    sb_Q[:, ftn, :], ps, sb_bq[:, ftn : ftn + 1]
                )

            def emit_V(nh, tt2):
                psv = ps_qkv.tile([128, 512], f32, tag="p", name="kq_ps")
                for kt in range(FT):
                    nc.tensor.matmul(
                        psv[:, 0:384],
                        xt_t[kt][:, tt2 * 128 : (tt2 + 1) * 128],
                        wv_t[kt][:, nh * 384 : (nh + 1) * 384],
                        start=(kt == 0),
                        stop=(kt == FT - 1),
                    )
                nc.vector.scalar_tensor_tensor(
                    out=sb_V[:, tt2, nh * 6 : (nh + 1) * 6, 0:DK],
                    in0=psv[:, 0:384].rearrange("p (h d) -> p h d", d=DK),
                    scalar=1.0,
                    in1=bv_bc[:, nh * 384 : (nh + 1) * 384].rearrange(
                        "p (h d) -> p h d", d=DK
                    ),
                    op0=ALU.mult,
                    op1=ALU.add,
                )

            # software-pipelined emission: projection chains of the NEXT
            # feature tile are fed into the current tile's score/exp/z
            # stream as filler so the in-order PE queue always has work
            # while the scalar engine chews on exps.
            fillers: list = []
            v_emitted = 0  # V chains of nh=0 emitted so far (need-before-z)

            # upfront: K/Q for ft0 and the first 4 nh=0 V chains
            for nt in range(4):
                emit_K(0, nt)
            emit_Q(0)
            while v_emitted < 4:
                emit_V(0, v_emitted)
                v_emitted += 1
            # remaining nh=0 V chains are force-drained just ahead of their
            # consuming z batch; everything else is bubble filler
            for tt2 in range(ST):
                fillers.append(("V1", tt2))
            for nt in range(4):
                fillers.append(("K", 1, nt))
            fillers.append(("Q", 1))

            def emit_filler(n):
                for _ in range(n):
                    if not fillers:
                        return
                    f = fillers.pop(0)
                    if f[0] == "V1":
                        emit_V(1, f[1])
                    elif f[0] == "K":
                        emit_K(f[1], f[2])
                    else:
                        emit_Q(f[1])

            for ft in range(FT):
                for half in (0, 1):
                    h = 2 * ft + half
                    ho = half * 64
                    zp = ps_z.tile([128, BLK], f32, tag="z")
                    emit_recip()
                    for b8 in range(ST // 2):
                        pssc = ps_sc.tile([128, 2, 512], f32, tag="sc")
                        last_sc = None
                        for j in (0, 1):
                            kt2 = b8 * 2 + j
                            last_sc = nc.tensor.matmul(
                                pssc[:, j, :],
                                sb_K[ho : ho + 64, ft, kt2 * 128 : (kt2 + 1) * 128],
                                sb_Q[ho : ho + 64, ft, :],
                                start=True,
                                stop=True,
                            )
                        # nh=0 V chains must land ahead of the z that reads them
                        if ft == 0 and half == 0:
                            while v_emitted < min(ST, 2 * b8 + 4):
                                emit_V(0, v_emitted)
                                v_emitted += 1
                        else:
                            emit_filler(2)
                        et = ets_p.tile([128, 2, BLK], bf16, tag="exp")
                        nc.scalar.activation(
                            et[:].rearrange("p a b -> p (a b)"),
                            pssc[:].rearrange("p a b -> p (a b)"),
                            AF.Exp,
                            scale=ISCALE,
                        )
                        for j in (0, 1):
                            kt2 = b8 * 2 + j
                            nc.tensor.matmul(
                                zp[0 : DK + 1, :],
                                sb_V[:, kt2, h, :],
                                et[:, j, :],
                                start=(kt2 == 0),
                                stop=(kt2 == ST - 1),
                            )
                        if b8 == 5:
                            # previous head's normalization tail goes here so
                            # the PE never waits on the DVE reciprocal
                            flush_tail(anchor=last_sc)
                    pending.append({"zp": zp, "ho": ho, "ft": ft})
                    # drain remaining fillers for the next tile before its
                    # scores need them
                    if half == 1:
                        emit_filler(len(fillers))
                        if ft + 1 < FT and ft + 2 <= FT - 1 or ft + 1 < FT:
                            pass
                # queue next tile's projections as filler for the following one
                if ft + 2 <= FT - 1:
                    for nt in range(4):
                        fillers.append(("K", ft + 2, nt))
                    fillers.append(("Q", ft + 2))

            emit_recip()
            flush_tail()

            if TAPS:
                nc.sync.dma_start(out=d_tap_zT[:], in_=sb_zT)

            wqkv_cm.__exit__(None, None, None)
            xt_cm.__exit__(None, None, None)
            ps_z_cm.__exit__(None, None, None)
            ps_sc_cm.__exit__(None, None, None)
            ps_qkv_cm.__exit__(None, None, None)
            attsm_cm.__exit__(None, None, None)
            ets_cm.__exit__(None, None, None)

            # ============ O proj + LN1 (+residual) + l1 transpose ============
            def layer_norm_to(out_ap, x_ap, g_bc_t, resid_ap, pool):
                # spread across Pool (reduce/sub, SBUF-only) / ACT (sq, sqrt)
                # / DVE (recip, fused scale+gain, final add)
                s = pool.tile([128, 1], f32, tag="ln_s")
                nc.vector.tensor_reduce(s, x_ap, axis=AX.X, op=ALU.add)
                mean = pool.tile([128, 1], f32, tag="ln_m")
                nc.vector.tensor_scalar_mul(mean, s, 1.0 / DIM)
                xc = pool.tile([128, DIM], f32, tag="ln_xc")
                nc.vector.tensor_scalar(xc, x_ap, mean, None, op0=ALU.subtract)
                junk = pool.tile([128, DIM], bf16, tag="ln_j")
                var = pool.tile([128, 1], f32, tag="ln_v")
                nc.scalar.activation(junk, xc, AF.Square, accum_out=var)
                sd = pool.tile([128, 1], f32, tag="ln_sd")
                nc.scalar.activation(sd, var, AF.Sqrt, bias=eps_t[:], scale=1.0 / DIM)
                rstd = pool.tile([128, 1], f32, tag="ln_r")
                nc.vector.reciprocal(rstd, sd)
                tg = pool.tile([128, DIM], f32, tag="ln_tg")
                nc.vector.scalar_tensor_tensor(
                    out=tg, in0=xc, scalar=rstd, in1=g_bc_t,
                    op0=ALU.mult, op1=ALU.mult,
                )
                nc.vector.tensor_add(out_ap, tg, resid_ap)

            xb_r = d_xb[:].rearrange("(t p) d -> p t d", p=128)
            with (
                tc.tile_pool(name="ln1p", bufs=2) as ln1p,
                tc.tile_pool(name="ps_o", bufs=2, space="PSUM") as ps_o,
                tc.tile_pool(name="ps_t", bufs=2, space="PSUM") as ps_t,
            ):
                def _o_psum():
                    t = ps_o.tile([128, 384], f32, tag="op", name="bc_o")
                    return t[:]

                pe_bcast(bo_bc, row_bo, DIM, _o_psum, chunk=384)
                pe_bcast(g1_bc, row_g1, DIM, _o_psum, chunk=384)
                pe_bcast(bb1_bc, row_bb1, DIM, _o_psum, chunk=384)
                for tt in range(TT):
                    l1pre = ln1p.tile([128, DIM], f32, tag="l1pre")
                    for nh in range(2):
                        ps = ps_o.tile([128, 384], f32, tag="op")
                        for kt in range(FT):
                            nc.tensor.matmul(
                                ps,
                                sb_zT[:, kt, tt * 128 : (tt + 1) * 128],
                                wo_t[kt][:, nh * 384 : (nh + 1) * 384],
                                start=(kt == 0),
                                stop=(kt == FT - 1),
                            )
                        nc.vector.scalar_tensor_tensor(
                            out=l1pre[:, nh * 384 : (nh + 1) * 384],
                            in0=ps,
                            scalar=1.0,
                            in1=bo_bc[:, nh * 384 : (nh + 1) * 384],
                            op0=ALU.mult,
                            op1=ALU.add,
                        )
                    xbt = ln1p.tile([128, DIM], f32, tag="xbt")
                    nc.sync.dma_start(out=xbt, in_=xb_r[:, tt, :])
                    xb1 = ln1p.tile([128, DIM], f32, tag="xb1")
                    nc.vector.tensor_add(xb1, xbt, bb1_bc)
                    layer_norm_to(sb_l1[:, tt, :], l1pre[:], g1_bc, xb1, ln1p)
                    # transpose l1[tt] right away so FFN1 can start early;
                    # PSUM->SBUF copies on ACT (idle after attention)
                    for ft in range(FT):
                        pst = ps_t.tile([128, 128], f32, tag="tp")
                        nc.tensor.transpose(
                            pst, sb_l1[:, tt, ft * 128 : (ft + 1) * 128], ident[:]
                        )
                        nc.scalar.copy(
                            sb_l1T[:, ft, tt * 128 : (tt + 1) * 128], pst
                        )

            if TAPS:
                nc.sync.dma_start(out=d_tap_l1[:], in_=sb_l1)

            wo_cm.__exit__(None, None, None)
            attn_cm.__exit__(None, None, None)

            # ============ FFN1 -> hT, FFN2 streamed behind it ============
            w1_cm = tc.tile_pool(name="w1_p", bufs=1)
            w1_p = w1_cm.__enter__()
            w1_t = []
            for kt in range(FT):
                w1t = w1_p.tile([128, HID], bf16, tag=f"w1_{kt}")
                q_dma(w1t, d_w1[kt * 128 : (kt + 1) * 128, :], HID * 128 * 2)
                w1_t.append(w1t)
            hT_cm = tc.tile_pool(name="hT_p", bufs=1)
            hT_p = hT_cm.__enter__()
            sb_hT = hT_p.tile([128, HT, BLK], bf16)  # relu(ffn1)^T, hid-major

            with (
                tc.tile_pool(name="w2s", bufs=1) as w2s_p,
                tc.tile_pool(name="ln2p", bufs=2) as ln2p,
                tc.tile_pool(name="f2pre_p", bufs=4) as f2pre_p,
                tc.tile_pool(name="outp", bufs=3) as outp,
                tc.tile_pool(name="ps_f1", bufs=2, space="PSUM") as ps_f1,
                tc.tile_pool(name="ps_f2", bufs=4, space="PSUM") as ps_f2,
            ):
                # prefetch all of w2 in 8 chunks (4 kt-groups x 2 halves)
                w2c = {}
                for nh in range(2):
                    for kg in range(4):
                        w2t = w2s_p.tile([128, 6, 384], bf16, tag=f"w2c{nh}{kg}")
                        q_dma(
                            w2t,
                            d_w2[
                                kg * 768 : (kg + 1) * 768,
                                nh * 384 : (nh + 1) * 384,
                            ].rearrange("(t p) c -> p t c", p=128),
                            6 * 384 * 128 * 2,
                        )
                        w2c[(nh, kg)] = w2t

                out_r = d_out[:].rearrange("(t p) d -> p t d", p=128)

                def _f2_psum():
                    t = ps_f2.tile([128, 384], f32, tag="f2", name="bc_f2")
                    return t[:]

                pe_bcast(b2_bc, row_b2, DIM, _f2_psum, chunk=384)
                pe_bcast(g2_bc, row_g2, DIM, _f2_psum, chunk=384)
                pe_bcast(bb2_bc, row_bb2, DIM, _f2_psum, chunk=384)

                def ffn1_half(hf):
                    # FFN1 for token half hf (cols hf*256 .. hf*256+256)
                    for ht2 in range(HT):
                        ps = ps_f1.tile([128, 256], f32, tag="f1")
                        for kt in range(FT):
                            nc.tensor.matmul(
                                ps,
                                w1_t[kt][:, ht2 * 128 : (ht2 + 1) * 128],
                                sb_l1T[:, kt, hf * 256 : (hf + 1) * 256],
                                start=(kt == 0),
                                stop=(kt == FT - 1),
                            )
                        # relu(x + b1) on DVE: (x add b1) max 0
                        nc.vector.tensor_scalar(
                            sb_hT[:, ht2, hf * 256 : (hf + 1) * 256], ps,
                            sb_b1[:, ht2 : ht2 + 1], 0.0,
                            op0=ALU.add, op1=ALU.max,
                        )

                def ffn2_tile(tt):
                    # both 384-col halves for one token tile, then LN2 + out
                    f2pre = f2pre_p.tile([128, DIM], f32, tag="f2pre")
                    for nh in range(2):
                        chain = ps_f2.tile([128, 384], f32, tag="f2")
                        for kt in range(HT):
                            nc.tensor.matmul(
                                chain,
                                sb_hT[:, kt, tt * 128 : (tt + 1) * 128],
                                w2c[(nh, kt // 6)][:, kt % 6, :],
                                start=(kt == 0),
                                stop=(kt == HT - 1),
                            )
                        nc.vector.scalar_tensor_tensor(
                            out=f2pre[:, nh * 384 : (nh + 1) * 384],
                            in0=chain,
                            scalar=1.0,
                            in1=b2_bc[:, nh * 384 : (nh + 1) * 384],
                            op0=ALU.mult,
                            op1=ALU.add,
                        )
                    l1b = ln2p.tile([128, DIM], f32, tag="l1b")
                    nc.vector.tensor_add(l1b, sb_l1[:, tt, :], bb2_bc)
                    o_sb = outp.tile([128, DIM], f32, tag="osb")
                    layer_norm_to(o_sb[:], f2pre[:], g2_bc, l1b, ln2p)
                    nc.sync.dma_start(out=out_r[:, tt, :], in_=o_sb)

                ffn1_half(0)
                ffn2_tile(0)
                ffn2_tile(1)
                ffn1_half(1)
                ffn2_tile(2)
                ffn2_tile(3)

            hT_cm.__exit__(None, None, None)
            w1_cm.__exit__(None, None, None)

    return nc


def _get_nc(finalized=True):
    if "nc" not in _CACHE:
        _CACHE["nc"] = _build_program()
    nc = _CACHE["nc"]
    if finalized and not nc.is_finalized():
        nc.finalize()
    return nc


def make_in_maps(inputs: dict) -> list:
    x = np.asarray(inputs["x_n"], np.float32).reshape(B, S, DIM)
    mask = np.asarray(inputs["mask"]).reshape(B, S)
    w = {
        k: np.ascontiguousarray(np.asarray(inputs[k], np.float32).astype(BF16))
        for k in ("wq", "wk", "wv", "wo", "w1", "w2")
    }
    vecs = {
        "bq": inputs["bq"], "bk": inputs["bk"], "bv": inputs["bv"],
        "bo": inputs["bo"], "b1": inputs["b1"], "b2": inputs["b2"],
        "g1": inputs["ln1_g"], "bb1": inputs["ln1_b"],
        "g2": inputs["ln2_g"], "bb2": inputs["ln2_b"],
    }
    vecs = {k: np.ascontiguousarray(np.asarray(v, np.float32)) for k, v in vecs.items()}
    in_maps = []
    for c in range(N_CORES):
        b, blk = c // NBLK, c % NBLK
        xb_full = x[b]
        xT = np.ascontiguousarray(xb_full.T.astype(BF16))
        xblk = np.ascontiguousarray(xb_full[blk * BLK : (blk + 1) * BLK])
        xTb = np.ascontiguousarray(xblk.T.astype(BF16))
        msk = (mask[b] != 0).astype(np.float32)
        m = {"xT": xT, "xTb": xTb, "xb": xblk, "msk": msk}
        m.update(w)
        m.update(vecs)
        in_maps.append(m)
    return in_maps


def assemble(per_core_out: list) -> np.ndarray:
    blocks = [np.asarray(o, np.float32) for o in per_core_out]
    full = np.concatenate(blocks, axis=0).reshape(B, S, DIM)
    return full


def kernel(**inputs) -> np.ndarray:
    from concourse.bass_utils import run_bass_kernel_spmd

    nc = _get_nc()
    in_maps = make_in_maps(inputs)
    res = run_bass_kernel_spmd(nc, in_maps, list(range(N_CORES)))
    return assemble([r["out"] for r in res.results])
